# revision 1
# baseline (speedup 1.0000x reference)
"""Banded causal self-attention (band width 64) on 8 trn2 NeuronCores.

Sequence-parallel sharding: core c handles batch c//4, query block c%4
(512 queries of T=2048), recomputing a 64-token k/v halo locally so no
collectives are needed. The host casts inputs to bf16 and transposes x
per core; the device kernel fuses qkv-projection -> banded attention ->
output projection.

Device layouts (per core):
  xt    [C, 576]      x chunk transposed (64-token halo + 512 owned)
  qk^T  [2048, 576]   q/k feature-major (slab h//2 (+8 for k), rows (h%2)*64)
  v     [576, 1024]   token-major
  y^T   [1024, 512]   attention output feature-major
  out   [512, 1024]   tokens x C

Attention is computed transposed (S^T[key, query] per 128-key chunk):
the exp(S^T) tile feeds the AV matmul directly as the moving operand, so
no PE transposes or PSUM round-trips are needed. The band mask is a
multiplicative {0,1} bf16 mask (2 static patterns). Rowsums come from a
voner-stationary matmul (host-zeroed on padded halo tokens) replicated
over 64 partitions so the reciprocal and normalize-multiply stay
partition-aligned with each head's half of the PSUM accumulator. Softmax skips max-subtraction (scores are O(1)).
"""

import numpy as np
import ml_dtypes

import concourse.mybir as mybir
import concourse.tile as tile
from concourse import bacc
from concourse import bass_utils

B, T, C, H, D = 2, 2048, 1024, 16, 64
W = 64            # band width: key j visible to query i iff i-64 <= j <= i
N_CORES = 8
QL = 512          # queries per core
HT = QL + W       # tokens incl. halo
P = 128
KC = C // P       # contraction chunks
NFT = 2 * C // P  # q|k feature slabs
NKC = 5           # key chunks (4x128 + 64)
VW = D            # v columns per head

bf16 = mybir.dt.bfloat16
f32 = mybir.dt.float32
Act = mybir.ActivationFunctionType

_CACHE = {}

# per key-chunk: (chunk keys, query-col start, query-col end, mask pattern)
CHUNKS = []
for c in range(NKC):
    kn = P if c < NKC - 1 else W
    cs = max(0, P * c - W)
    ce = min(QL, P * c + P)
    CHUNKS.append((kn, cs, ce, 0 if c == 0 else 1))


def _emit(tc, xt, wqk, wv, wp, bqk, bvr, bvr0, bpr, maskT, voner, out):
    nc = tc.nc
    with (
        tc.tile_pool(name="const", bufs=1) as const,
        tc.tile_pool(name="wqkp", bufs=3) as wqkp,
        tc.tile_pool(name="attn", bufs=4) as at,
        tc.tile_pool(name="ot", bufs=3) as ot,
        tc.tile_pool(name="psA", bufs=2, space="PSUM") as psA,
        tc.tile_pool(name="psS", bufs=2, space="PSUM") as psSp,
        tc.tile_pool(name="psY", bufs=2, space="PSUM") as psYp,
        tc.tile_pool(name="psR", bufs=2, space="PSUM") as psRp,
    ):
        # ---- persistent tiles ----
        xt_sb = const.tile([P, KC, HT], bf16)
        nc.sync.dma_start(xt_sb[:], xt.rearrange("(kc p) t -> p kc t", p=P))
        wv_sb = const.tile([P, KC, C], bf16)
        wp_sb = const.tile([P, KC, C], bf16)
        maskT_sb = const.tile([P, 2, P + W], bf16)
        nc.sync.dma_start(maskT_sb[:], maskT.rearrange("m p k -> p m k"))
        bqk_sb = const.tile([P, NFT], f32)
        nc.sync.dma_start(bqk_sb[:], bqk.rearrange("(ft p) -> p ft", p=P))
        bvr_sb = const.tile([P, C], f32)
        nc.sync.dma_start(bvr_sb[:], bvr[:])
        bvr0_sb = const.tile([P, C], f32)
        nc.sync.dma_start(bvr0_sb[:], bvr0[:])
        bpr_sb = const.tile([P, C], f32)
        nc.sync.dma_start(bpr_sb[:], bpr[:])
        voner_sb = const.tile([P, NKC, D], bf16)
        nc.sync.dma_start(voner_sb[:, :NKC - 1], voner[:QL].rearrange("(c p) e -> p c e", p=P))
        nc.sync.dma_start(voner_sb[:W, NKC - 1], voner[QL:])
        zero_sb = const.tile([P, P], bf16)
        nc.gpsimd.memset(zero_sb[:], 0.0)

        qkT_sb = const.tile([P, NFT, HT], bf16)
        v_sb = const.tile([P, NKC, H * VW], bf16)
        yT_sb = const.tile([P, KC, QL], bf16)

        # ---- phase 1a: qk^T = Wqk^T @ x^T (feature-major) ----
        for ft in range(NFT):
            wt = wqkp.tile([P, KC, P], bf16, tag="wqk")
            nc.sync.dma_start(
                wt[:],
                wqk[:, ft * P:(ft + 1) * P].rearrange("(kc p) f -> p kc f", p=P),
            )
            # q is only needed for owned tokens (64:576); k for all 576
            segs = ((W, QL),) if ft < KC else ((0, QL), (QL, W))
            for t0, tsz in segs:
                psf = psA.tile([P, QL], f32, tag="mm", name="ps1a")
                ps = psf[:, :tsz]
                for kc in range(KC):
                    nc.tensor.matmul(
                        ps, wt[:, kc], xt_sb[:, kc, t0:t0 + tsz],
                        start=(kc == 0), stop=(kc == KC - 1),
                    )
                nc.scalar.activation(
                    qkT_sb[:, ft, t0:t0 + tsz], ps, Act.Identity,
                    bias=bqk_sb[:, ft:ft + 1],
                )

        # ---- phase 1b: v = x @ Wv (token-major) ----
        nc.sync.dma_start(wv_sb[:], wv.rearrange("(kc p) n -> p kc n", p=P))
        for tt in range(NKC):
            tsz = P if tt < NKC - 1 else W
            v_view = v_sb[:, tt].rearrange("p (h e) -> p h e", e=VW)
            bsel = bvr0_sb if tt == 0 else bvr_sb
            for hb, n0 in ((0, 0), (KC, QL)):
                psf = psA.tile([P, QL], f32, tag="mm", name="ps1b")
                ps = psf[:tsz]
                for kc in range(KC):
                    nc.tensor.matmul(
                        ps, xt_sb[:, kc, tt * P:tt * P + tsz],
                        wv_sb[:, kc, n0:n0 + QL],
                        start=(kc == 0), stop=(kc == KC - 1),
                    )
                nc.vector.tensor_add(
                    v_view[:tsz, hb:hb + KC, :],
                    ps.rearrange("p (h e) -> p h e", e=D),
                    bsel[:tsz, n0:n0 + QL].rearrange("p (h e) -> p h e", e=D),
                )

        # ---- phase 2: banded attention, transposed-S form ----
        nc.sync.dma_start(wp_sb[:], wp.rearrange("(kc p) n -> p kc n", p=P))
        for hp in range(H // 2):
            yA = psYp.tile([P, QL], f32, tag="yA")
            rs = psRp.tile([P, QL], f32, tag="rs")
            nc.tensor.matmul(yA[:], zero_sb[:], xt_sb[:, 0, 0:QL],
                             start=True, stop=False, skip_group_check=True)
            nc.tensor.matmul(rs[:], zero_sb[:], xt_sb[:, 0, 0:QL],
                             start=True, stop=False, skip_group_check=True)
            for c, (kn, cs, ce, mi) in enumerate(CHUNKS):
                wc = ce - cs
                psS = psSp.tile([P, 2, P + W], f32, tag="psS", name="psS")
                Pe = at.tile([P, 2, P + W], bf16, tag="Pe", name="Pe")
                for s in (0, 1):
                    r0 = D * s
                    nc.tensor.matmul(
                        psS[:kn, s, :wc],
                        qkT_sb[r0:r0 + D, KC + hp, c * P:c * P + kn],
                        qkT_sb[r0:r0 + D, hp, W + cs:W + ce],
                        start=True, stop=True,
                    )
                    nc.scalar.activation(Pe[:kn, s, :wc], psS[:kn, s, :wc],
                                         Act.Exp, scale=0.125)
                    nc.vector.tensor_mul(Pe[:kn, s, :wc], Pe[:kn, s, :wc],
                                         maskT_sb[:kn, mi, :wc])
                for s in (0, 1):
                    h = 2 * hp + s
                    r0 = D * s
                    nc.tensor.matmul(
                        yA[r0:r0 + D, cs:ce],
                        v_sb[:kn, c, h * VW:h * VW + D],
                        Pe[:kn, s, :wc],
                        start=False, stop=(s == 1 and c == NKC - 1),
                        tile_position=(0, r0), skip_group_check=True,
                    )
                    nc.tensor.matmul(
                        rs[r0:r0 + D, cs:ce],
                        voner_sb[:kn, c], Pe[:kn, s, :wc],
                        start=False, stop=(s == 1 and c == NKC - 1),
                        tile_position=(0, r0), skip_group_check=True,
                    )
            rr = at.tile([P, QL], f32, tag="rr", name="rr")
            nc.vector.reciprocal(rr[:], rs[:])
            for s in (0, 1):
                r0 = D * s
                nc.vector.tensor_mul(yT_sb[r0:r0 + D, hp, :],
                                     yA[r0:r0 + D], rr[r0:r0 + D])

        # ---- phase 3: out = y @ Wproj + b ----
        for tt in range(QL // P):
            for n0 in (0, QL):
                ps = psA.tile([P, QL], f32, tag="mm", name="ps3")
                for kc in range(KC):
                    nc.tensor.matmul(
                        ps, yT_sb[:, kc, tt * P:(tt + 1) * P],
                        wp_sb[:, kc, n0:n0 + QL],
                        start=(kc == 0), stop=(kc == KC - 1),
                    )
                osb = ot.tile([P, QL], f32, tag="osb", name="osb")
                nc.vector.tensor_add(osb[:], ps, bpr_sb[:, n0:n0 + QL])
                nc.sync.dma_start(out[tt * P:(tt + 1) * P, n0:n0 + QL], osb[:])


def _build():
    nc = bacc.Bacc(
        "TRN2", target_bir_lowering=False, debug=False,
        enable_asserts=True, num_devices=N_CORES,
    )
    xt = nc.dram_tensor("xt", [C, HT], bf16, kind="ExternalInput").ap()
    wqk = nc.dram_tensor("wqk", [C, 2 * C], bf16, kind="ExternalInput").ap()
    wv = nc.dram_tensor("wv", [C, C], bf16, kind="ExternalInput").ap()
    wp = nc.dram_tensor("wp", [C, C], bf16, kind="ExternalInput").ap()
    bqk = nc.dram_tensor("bqk", [2 * C], f32, kind="ExternalInput").ap()
    bvr = nc.dram_tensor("bvr", [P, C], f32, kind="ExternalInput").ap()
    bvr0 = nc.dram_tensor("bvr0", [P, C], f32, kind="ExternalInput").ap()
    bpr = nc.dram_tensor("bpr", [P, C], f32, kind="ExternalInput").ap()
    maskT = nc.dram_tensor("maskT", [2, P, P + W], bf16, kind="ExternalInput").ap()
    voner = nc.dram_tensor("voner", [HT, D], bf16, kind="ExternalInput").ap()
    out = nc.dram_tensor("out", [QL, C], f32, kind="ExternalOutput").ap()
    with tile.TileContext(nc) as tc:
        _emit(tc, xt, wqk, wv, wp, bqk, bvr, bvr0, bpr, maskT, voner, out)
    nc.compile()
    return nc


def _get_module():
    if "nc" not in _CACHE:
        _CACHE["nc"] = _build()
    return _CACHE["nc"]


def _band_masks() -> np.ndarray:
    # pattern 0 (chunk 0):  keep iff  y <= p <= y+64
    # pattern 1 (chunks>0): keep iff  y-64 <= p <= y
    p = np.arange(P)[:, None]
    y = np.arange(P + W)[None, :]
    m0 = (p >= y) & (p <= y + W)
    m1 = (p >= y - W) & (p <= y)
    return np.stack([m0, m1]).astype(ml_dtypes.bfloat16)


def kernel(x, Wqkv, bqkv, Wproj, bproj):
    x = np.asarray(x, dtype=np.float32)
    Wqkv = np.asarray(Wqkv, dtype=np.float32)
    bqkv = np.asarray(bqkv, dtype=np.float32)
    Wproj = np.asarray(Wproj, dtype=np.float32)
    bproj = np.asarray(bproj, dtype=np.float32)

    bf = ml_dtypes.bfloat16
    wqk_np = np.ascontiguousarray(Wqkv[:, :2 * C]).astype(bf)
    wv_np = np.ascontiguousarray(Wqkv[:, 2 * C:]).astype(bf)
    wp_np = Wproj.astype(bf)
    bqk_np = np.ascontiguousarray(bqkv[:2 * C])
    bvr_np = np.ascontiguousarray(np.broadcast_to(bqkv[2 * C:], (P, C)))
    bvr0_np = bvr_np.copy()
    bvr0_np[:W] = 0.0  # halo-pad tokens of the q==0 cores carry no bias
    bpr_np = np.ascontiguousarray(np.broadcast_to(bproj, (P, C)))
    maskT_np = _band_masks()

    vone_real = np.ones((HT, D), dtype=bf)
    vone_pad = vone_real.copy()
    vone_pad[:W] = 0.0

    in_maps = _prep_in_maps(x, wqk_np, wv_np, wp_np, bqk_np, bvr_np, bvr0_np,
                            bpr_np, maskT_np, vone_real, vone_pad)

    nc = _get_module()
    _CACHE["last_in_maps"] = in_maps
    res = bass_utils.run_bass_kernel_spmd(nc, in_maps, core_ids=list(range(N_CORES)))

    out = np.empty((B, T, C), dtype=np.float32)
    for c in range(N_CORES):
        b, q = divmod(c, 4)
        out[b, q * QL:(q + 1) * QL] = res.results[c]["out"]
    return out


def _prep_in_maps(x, wqk_np, wv_np, wp_np, bqk_np, bvr_np, bvr0_np, bpr_np,
                  maskT_np, vone_real, vone_pad):
    bf = ml_dtypes.bfloat16
    in_maps = []
    for c in range(N_CORES):
        b, q = divmod(c, 4)
        lo = q * QL - W
        if lo < 0:
            chunk = np.concatenate(
                [np.zeros((W, C), np.float32), x[b, 0:q * QL + QL]], axis=0
            )
        else:
            chunk = x[b, lo:lo + HT]
        in_maps.append({
            "xt": np.ascontiguousarray(chunk.T).astype(bf),
            "wqk": wqk_np,
            "wv": wv_np,
            "wp": wp_np,
            "bqk": bqk_np,
            "bvr": bvr_np,
            "bvr0": bvr0_np if q == 0 else bvr_np,
            "bpr": bpr_np,
            "maskT": maskT_np,
            "voner": vone_pad if q == 0 else vone_real,
        })
    return in_maps

